# revision 28
# baseline (speedup 1.0000x reference)
"""TRN2 Bass kernel for nn_CrossModalAttention_75316546503126.

Mathematical collapse (verified against the jax reference): the acoustic
features are broadcast along the sequence axis BEFORE the K/V projections,
so every attention row sees an identical score vector; softmax of a
constant row is exactly uniform (S=2048 is a power of two, 1/S exact in
fp32) and the attention output is the per-batch V row:

    v_b           = (ac_b @ Wa + ba) @ Wv + bv
    out[b, s, :]  = text[b, s, :] @ Wt + (bt + v_b)

Q/K projections cancel entirely.  The device work per batch is ONE
[2048, 768] @ [768, 768] matmul plus a broadcast bias row.

Sharding: data-parallel over batch B=8, one NeuronCore per batch.

Device strategy (per core, bf16):
  * x is transposed + cast to bf16 on the host, so the PE does ZERO
    transposes: x^T kblocks land in SBUF with K on partitions and serve
    directly as the matmul stationary.
  * Wt bf16 (1 cycle/row PE rate, half the DMA bytes of fp32); the tiny
    bias row bt+bv+fa@Wv is host-computed and DMA'd pre-broadcast.
  * Output written bf16, upcast to fp32 on the host.  End-to-end
    max-rel error vs the fp32 reference ~3.7e-3 (gate is 2e-2).

Schedule (learned from the NTFF trace of v1):
  * DMA triggers cost ~0.7us serialized on the issuing engine, so inputs
    use only 7 triggers spread over Sync/Vector/Scalar engines.
  * x streams in 4 chunks (256/512/640/640 cols) — a small first chunk
    so the first matmul starts ~3us earlier; later chunks have >=1KB
    descriptor lines for full DMA rate.
  * 8 zero-data warmup matmuls run while the first DMAs are in flight:
    they start the HAM p-state ramp (~3us to 2.4GHz), so the real 192
    matmuls all run at the full 0.42ns/row rate.
  * DVE fuses the bias add into each PSUM->SBUF eviction; the last two
    tiles store singly so the final evict->store chain is short.
"""
import sys

if "/opt/trn_rl_repo" not in sys.path:
    sys.path.insert(0, "/opt/trn_rl_repo")

from contextlib import ExitStack

import numpy as np
import ml_dtypes

import concourse.bacc as bacc
import concourse.mybir as mybir
import concourse.tile as tile
from concourse.bass_utils import run_bass_kernel_spmd

F32 = mybir.dt.float32
BF16 = mybir.dt.bfloat16
NP_BF16 = ml_dtypes.bfloat16

B, S, D = 8, 2048, 768
KB = D // 128              # 6 contraction kblocks
ST = S // 128              # 16 sequence tiles per core
CB = [0, 256, 512, 768, 896, 1408, 2048]   # x chunk column boundaries
N_CORES = 8
N_WARM = 11


def build_program():
    nc = bacc.Bacc()

    xt = nc.declare_dram_parameter("xt", [D, S], BF16, isOutput=False)
    wt = nc.declare_dram_parameter("wt", [D, D], BF16, isOutput=False)
    bias = nc.declare_dram_parameter("bias", [128, D], F32, isOutput=False)
    out = nc.declare_dram_parameter("out", [S, D], BF16, isOutput=True)

    with tile.TileContext(nc) as tc, ExitStack() as ctx:
        wpool = ctx.enter_context(tc.tile_pool(name="wpool", bufs=1))
        xpool = ctx.enter_context(tc.tile_pool(name="xpool", bufs=1))
        cpool = ctx.enter_context(tc.tile_pool(name="cpool", bufs=1))
        opool = ctx.enter_context(tc.tile_pool(name="opool", bufs=2))
        psum = ctx.enter_context(tc.tile_pool(name="psum", bufs=3, space="PSUM"))
        wps = ctx.enter_context(tc.tile_pool(name="wps", bufs=1, space="PSUM"))

        # ---- warmup: PE p-state ramp while first DMAs fly ----
        warm = cpool.tile([128, 512], BF16, name="warm")
        nc.gpsimd.memset(warm[:], 0.0)
        warm_po = wps.tile([128, 512], F32, name="warm_po")
        for _ in range(N_WARM):
            nc.tensor.matmul(warm_po[:], warm[:, 0:128], warm[:],
                             start=True, stop=True)

        # ---- input DMAs: few triggers, spread across engines ----
        xchunks = []
        for c in range(len(CB) - 1):
            w = CB[c + 1] - CB[c]
            t = xpool.tile([128, KB * w], BF16, tag=f"x{c}", name=f"x{c}")
            nc.sync.dma_start(
                t[:].rearrange("p (k s) -> p k s", k=KB),
                xt[:, CB[c]:CB[c + 1]].rearrange("(k p) s -> p k s", p=128),
            )
            xchunks.append(t)

        # weights in 3 DMAs so early kblocks unlock before the full 1.2MB
        # lands (every tile needs all of Wt — this gates the whole stream)
        wt0 = wpool.tile([128, D], BF16, tag="wt0", name="wt0")
        nc.scalar.dma_start(wt0[:], wt[0:128, :])
        wta = wpool.tile([128, 2 * D], BF16, tag="wta", name="wta")
        nc.scalar.dma_start(
            wta[:].rearrange("p (k d) -> p k d", k=2),
            wt[128:384, :].rearrange("(k p) d -> p k d", p=128),
        )
        wtb1 = wpool.tile([128, D], BF16, tag="wtb1", name="wtb1")
        nc.scalar.dma_start(wtb1[:], wt[384:512, :])
        wtb2 = wpool.tile([128, 2 * D], BF16, tag="wtb2", name="wtb2")
        nc.scalar.dma_start(
            wtb2[:].rearrange("p (k d) -> p k d", k=2),
            wt[512:D, :].rearrange("(k p) d -> p k d", p=128),
        )
        bias_sb = cpool.tile([128, D], F32, name="bias_sb")
        nc.scalar.dma_start(bias_sb[:], bias[:])

        def wslice(k, lo, hi):
            if k == 0:
                return wt0[:, lo:hi]
            if k <= 2:
                return wta[:, (k - 1) * D + lo:(k - 1) * D + hi]
            if k == 3:
                return wtb1[:, lo:hi]
            return wtb2[:, (k - 4) * D + lo:(k - 4) * D + hi]

        def lhs_ap(i, k):
            c = 0
            while CB[c + 1] <= i * 128:
                c += 1
            w = CB[c + 1] - CB[c]
            off = i * 128 - CB[c]
            return xchunks[c][:, k * w + off:k * w + off + 128]

        def emit_k(i, po, ks):
            for k in ks:
                lhs = lhs_ap(i, k)
                st, sp = (k == 0), (k == KB - 1)
                nc.tensor.matmul(po[:, 0:512], lhs, wslice(k, 0, 512),
                                 start=st, stop=sp)
                nc.tensor.matmul(po[:, 512:D], lhs, wslice(k, 512, D),
                                 start=st, stop=sp)

        # ---- main loop ----
        osb = [None]

        def finish_tile(i, po):
            """Evict po (+bias) to SBUF bf16 and store per 2-tile pair."""
            j = i % 2
            if i < ST - 2:
                if j == 0:
                    osb[0] = opool.tile([128, 2 * D], BF16, tag="osb",
                                        name=f"osb{i // 2}")
                nc.vector.tensor_add(osb[0][:, j * D:(j + 1) * D], po[:],
                                     bias_sb[:])
                if j == 1:
                    sb = i // 2
                    nc.scalar.dma_start(
                        out[sb * 256:(sb + 1) * 256, :].rearrange(
                            "(j p) d -> p j d", p=128),
                        osb[0][:].rearrange("p (j d) -> p j d", j=2),
                    )
            else:  # last two tiles: single stores so the tail chain is short
                ot = opool.tile([128, D], BF16, tag="otail", name=f"ot{i}")
                nc.vector.tensor_add(ot[:], po[:], bias_sb[:])
                nc.scalar.dma_start(out[i * 128:(i + 1) * 128, :], ot[:])

        # tiles 0..2: kblocks 0-2 only, keeping the PE busy while the
        # wtb (k3-5) DMA is still in flight; finish + evict them once it
        # lands, then stream tiles 3..15 normally
        pos = []
        for i in range(3):
            po = psum.tile([128, D], F32, tag="po")
            emit_k(i, po, range(3))
            pos.append(po)
        for i in range(3):
            emit_k(i, pos[i], range(3, KB))
            finish_tile(i, pos[i])

        for i in range(3, ST):
            po = psum.tile([128, D], F32, tag="po")
            emit_k(i, po, range(KB))
            finish_tile(i, po)

    nc.compile()
    return nc


_PROGRAM_CACHE = {}


def _get_program():
    if "prog" not in _PROGRAM_CACHE:
        _PROGRAM_CACHE["prog"] = build_program()
    return _PROGRAM_CACHE["prog"]


def prepare_in_maps(text_features, acoustic_features, Wt, bt, Wa, ba,
                    Wv, bv, **_unused):
    """Host-side prep: per-batch bias row (tiny), x transpose + bf16 cast."""
    x = np.asarray(text_features, dtype=np.float32)
    ac = np.asarray(acoustic_features, dtype=np.float32)
    Wt = np.asarray(Wt, dtype=np.float32)
    Wa = np.asarray(Wa, dtype=np.float32)
    Wv = np.asarray(Wv, dtype=np.float32)
    bt = np.asarray(bt, dtype=np.float32)
    ba = np.asarray(ba, dtype=np.float32)
    bv = np.asarray(bv, dtype=np.float32)

    # bias_b = bt + bv + ((ac_b @ Wa) + ba) @ Wv     [B, D]
    fa = ac @ Wa + ba
    bias_rows = (bt + bv + fa @ Wv).astype(np.float32)

    wt_bf = np.ascontiguousarray(Wt.astype(NP_BF16))

    in_maps = []
    for b in range(N_CORES):
        m = {
            "xt": x[b].T.astype(NP_BF16),          # [D, S] contiguous
            "wt": wt_bf,
            "bias": np.ascontiguousarray(
                np.broadcast_to(bias_rows[b], (128, D))),
        }
        in_maps.append(m)
    return in_maps


def kernel(text_features, acoustic_features, Wt, bt, Wa, ba, Wq, bq, Wk, bk,
           Wv, bv, **_unused):
    nc = _get_program()
    in_maps = prepare_in_maps(text_features, acoustic_features, Wt, bt,
                              Wa, ba, Wv, bv)
    res = run_bass_kernel_spmd(nc, in_maps, list(range(N_CORES))).results
    out = np.empty((B, S, D), dtype=np.float32)
    for b in range(N_CORES):
        out[b] = res[b]["out"].astype(np.float32)
    return out
